# revision 13
# baseline (speedup 1.0000x reference)
"""Trainium2 Bass kernel for nn_MemoryCore (retrieval KNN min-distance).

Problem: embedding [8192, 512], memory_bank [65536, 512] (fp32) ->
patch_scores [8192, 1] = min over the bank of euclidean distance.

Strategy (8 NeuronCores, SPMD):
  - Shard the memory bank (M axis) 8 ways; every core sees all queries.
  - fp8(e4m3) DoubleRow matmuls (contraction 256/instr, 2 fp8/cycle stream
    = the fp8 PE peak): psum[m, n] = (-2*bank_shard) @ emb.T. Bank tile
    stationary; each weight feeds 8 matmuls (2 query blocks x 2 groups)
    into two 2-bank psum tiles [128, 1024] (4 bufs).
  - Query groups are processed in PAIRS so the V-path bf16 min runs at
    FD=2048 across both groups' adjacent query columns.
  - PSUM evacuation is split so neither engine exceeds the PE's ~440us of
    matmul streaming (measured: stt 1283ns, ACTIVATE 1111ns, TT@2048
    ~1223ns):
      D: DVE rm = min(psum + m_sq[m], rm)      (fused stt, 1x from PSUM)
      V: ACT tmp = bf16(psum + m_sq[m]); DVE rm = min(tmp, rm) (2x bf16)
    bf16 mins are emitted LAG units late so a slow ACT never blocks
    PSUM-critical stt ops at the head of the DVE's strict FIFO.
  - No device epilogue: ship the bf16 running mins; host does the
    cross-partition + cross-core min, adds ||x||^2, sqrt.
"""
import numpy as np
import ml_dtypes
import concourse.bacc as bacc
import concourse.mybir as mybir
import concourse.tile as tile
from concourse.bass_utils import run_bass_kernel_spmd

N_CORES = 8
N, M, D = 8192, 65536, 512
MS = M // N_CORES       # 8192 bank rows per core
MT = MS // 128          # 64 bank tiles (psum partition dim)
G = 2                   # query blocks (512 each) per psum tile
GW = 512 * G            # 1024 queries per group
NPAIR = N // (2 * GW)   # 4 group pairs (2048 queries each)
BIG = 1e30
DT = mybir.dt.float8e4  # e4m3 (TRN variant, max +-240): 2x PE with DoubleRow
# per-mt evacuation path (both groups of the pair take the same path):
# 9/32 D + 23/32 V balances DVE (~410us) against ACT (~410us).
PATTERN = "DVVVDVVDVVVDVVDVVVDVVDVVVDVVDVVV"
LAG = 4

_CACHE = {}


def _build_kernel():
    nc = bacc.Bacc("TRN2", target_bir_lowering=False, debug=False,
                   num_devices=N_CORES)

    embT_d = nc.dram_tensor("embT", [D, N], DT, kind="ExternalInput")
    bankT_d = nc.dram_tensor("bankT", [D, MS], DT, kind="ExternalInput")
    msq_d = nc.dram_tensor("msq", [128, MT], mybir.dt.float32, kind="ExternalInput")
    outv_d = nc.dram_tensor("outv", [128, N], mybir.dt.bfloat16,
                            kind="ExternalOutput")

    PW = 2 * GW  # 2048 queries per group pair

    with tile.TileContext(nc) as tc:
        with (
            tc.tile_pool(name="persist", bufs=1) as persist,
            tc.tile_pool(name="tmp", bufs=8) as tmpp,
            tc.tile_pool(name="psum", bufs=4, space="PSUM") as psum,
        ):
            msq = persist.tile([128, MT], mybir.dt.float32, tag="msq")
            nc.sync.dma_start(msq[:], msq_d[:])

            bank_t = persist.tile([128, 4, MS], DT, tag="bank")
            emb_t = persist.tile([128, 4, N], DT, tag="emb")
            # bank chunks on the sync queue; emb per-pair column slices on
            # the gpsimd queue so both rings run in parallel and pair 0's
            # matmuls start after ~5MB, not 8MB.
            for k in range(4):
                nc.sync.dma_start(bank_t[:, k, :],
                                  bankT_d[k * 128:(k + 1) * 128, :])
            for h in range(NPAIR):
                for k in range(4):
                    nc.gpsimd.dma_start(
                        emb_t[:, k, h * PW:(h + 1) * PW],
                        embT_d[k * 128:(k + 1) * 128, h * PW:(h + 1) * PW])

            rm_t = [persist.tile([128, PW], mybir.dt.bfloat16,
                                 name=f"rm{h}", tag=f"rm{h}")
                    for h in range(NPAIR)]

            for h in range(NPAIR):
                rm = rm_t[h]
                nc.gpsimd.memset(rm[:], BIG)
                pending = []
                for mt in range(MT):
                    psA = psum.tile([128, GW], mybir.dt.float32, tag="ps",
                                    name="psA")
                    psB = psum.tile([128, GW], mybir.dt.float32, tag="ps",
                                    name="psB")
                    for kp in range(2):
                        w = bank_t[:, kp * 2:(kp + 1) * 2,
                                   mt * 128:(mt + 1) * 128]
                        for u, ps_u in enumerate((psA, psB)):
                            g = 2 * h + u
                            for j in range(G):
                                nb = g * G + j
                                nc.tensor.matmul(
                                    ps_u[:, j * 512:(j + 1) * 512],
                                    w,
                                    emb_t[:, kp * 2:(kp + 1) * 2,
                                          nb * 512:(nb + 1) * 512],
                                    start=(kp == 0),
                                    stop=(kp == 1),
                                    perf_mode=mybir.MatmulPerfMode.DoubleRow,
                                )
                    if PATTERN[mt % len(PATTERN)] == "D":
                        # rm = min(psum + m_sq[m], rm)  (DVE, 1x from PSUM)
                        for u, ps_u in enumerate((psA, psB)):
                            nc.vector.scalar_tensor_tensor(
                                out=rm[:, u * GW:(u + 1) * GW],
                                in0=ps_u[:],
                                scalar=msq[:, mt:mt + 1],
                                in1=rm[:, u * GW:(u + 1) * GW],
                                op0=mybir.AluOpType.add,
                                op1=mybir.AluOpType.min,
                            )
                    else:
                        # ACT evacuates both halves (+ m_sq, downcast bf16)
                        t = tmpp.tile([128, PW], mybir.dt.bfloat16, tag="t")
                        for u, ps_u in enumerate((psA, psB)):
                            nc.scalar.activation(
                                out=t[:, u * GW:(u + 1) * GW], in_=ps_u[:],
                                func=mybir.ActivationFunctionType.Identity,
                                bias=msq[:, mt:mt + 1],
                            )
                        pending.append(t)
                    while len(pending) > LAG:
                        t = pending.pop(0)
                        nc.vector.tensor_tensor(
                            out=rm[:], in0=t[:], in1=rm[:],
                            op=mybir.AluOpType.min)
                for t in pending:
                    nc.vector.tensor_tensor(
                        out=rm[:], in0=t[:], in1=rm[:],
                        op=mybir.AluOpType.min)
                nc.sync.dma_start(outv_d[:, h * PW:(h + 1) * PW], rm[:])

    nc.compile()
    return nc


def kernel(embedding: np.ndarray, memory_bank: np.ndarray) -> np.ndarray:
    emb = np.asarray(embedding, dtype=np.float32)
    bank = np.asarray(memory_bank, dtype=np.float32)
    assert emb.shape == (N, D) and bank.shape == (M, D)

    if "nc" not in _CACHE:
        _CACHE["nc"] = _build_kernel()
    nc = _CACHE["nc"]

    embT8 = np.ascontiguousarray(emb.T).astype(ml_dtypes.float8_e4m3)
    x_sq = np.einsum("nd,nd->n", emb, emb, dtype=np.float64)  # [N]

    in_maps = []
    for c in range(N_CORES):
        shard = bank[c * MS:(c + 1) * MS]
        bankT8 = np.ascontiguousarray((-2.0 * shard).T).astype(
            ml_dtypes.float8_e4m3)
        m_sq = np.einsum("md,md->m", shard, shard,
                         dtype=np.float64).astype(np.float32)
        msq = np.ascontiguousarray(m_sq.reshape(MT, 128).T)
        in_maps.append({"embT": embT8, "bankT": bankT8, "msq": msq})

    _CACHE["last_in_maps"] = in_maps
    try:
        res = run_bass_kernel_spmd(nc, in_maps, core_ids=list(range(N_CORES)))
    except Exception:
        # a previously-wedged NeuronCore reports unrecoverable once and then
        # recovers; one retry clears it
        import time
        time.sleep(2.0)
        res = run_bass_kernel_spmd(nc, in_maps, core_ids=list(range(N_CORES)))

    # gather: each core returns [128, N] bf16 partial mins of (m_sq - 2 x.m);
    # min over partitions and cores, then + ||x||^2 and sqrt on host.
    per_core = np.stack([
        res.results[c]["outv"].astype(np.float64).min(axis=0)
        for c in range(N_CORES)
    ])  # [8, N]
    tot = per_core.min(axis=0) + x_sq
    return np.sqrt(np.maximum(tot, 0.0)).astype(np.float32).reshape(N, 1)
